# revision 50
# baseline (speedup 1.0000x reference)
"""Trainium2 Bass kernel for nn_AttentionBlock (GroupNorm + 1x1-conv QKV +
full self-attention over N=HW=4096 + output projection + residual).

Distribution: data-parallel over batch B=8, one batch element per NeuronCore.

v2: all matmul operands in bf16 (fp32 matmuls stream at ~2 cyc/col on TRN2
hardware -- "fp32_mode=HIGH/LOW" dual pass -- so bf16 doubles PE throughput).
The loop processes queries in 4 blocks of 1024 so exp runs as a single
[128,1024] ACT instruction per j-tile, with double-buffered S tiles in PSUM:

  per (block, j):  S^T = K_j^T Q_blk   (PE, 2x 512-col matmuls, bf16)
                   P^T = exp(S^T)      (ACT, PSUM->SBUF bf16)
                   O  += V_j^T P^T     (PE, 2x 512-col, PSUM accum over j)
                   acc += P^T          (DVE, bf16 pair tree)

Softmax denominators: bf16 pair-tree accumulation on DVE, cross-partition
sum via gpsimd partition_all_reduce (idle Pool engine), reciprocal on a
DMA-scattered [128,8] layout, broadcast back via gpsimd partition_broadcast.
Tail uses proj/normalize commutation: O is normalized per-query BEFORE the
projection (they commute since the denominator is a per-query scalar), so
out = (w_proj^T (O * recip)) + (x + b_eff) with the residual pre-folded.

Bias algebra: b_k folded into the K PSUM->SBUF copies (ACT bias); b_q into
the Q copies (DVE); b_v folded into b_eff = b_proj + w_proj @ b_v (host,
exact). The attention scale C^-0.5 is folded into w_q/b_q on the host.
No max-subtraction in softmax: logits are ~N(0,1) so fp32 exp is safe.
"""

import numpy as np

B, C, H, W = 8, 128, 64, 64
HW = H * W                      # 4096
GROUPS = 8
GSIZE = C // GROUPS             # 16
EPS = 1e-5
NJ = HW // 128                  # 32 j-tiles
QW = 1024                       # queries per block
NQT = HW // QW                  # 4 blocks
SCALE = float(C) ** -0.5

_CACHE = {}


def _build():
    from contextlib import ExitStack

    import concourse.bacc as bacc
    import concourse.tile as tile
    from concourse import bass_isa, mybir

    f32 = mybir.dt.float32
    bf16 = mybir.dt.bfloat16
    AF = mybir.ActivationFunctionType

    nc = bacc.Bacc("TRN2", target_bir_lowering=False, debug=False)

    x_in = nc.dram_tensor("x", [C, HW], f32, kind="ExternalInput")
    gamma_in = nc.dram_tensor("gamma", [C, 1], f32, kind="ExternalInput")
    beta_in = nc.dram_tensor("beta", [C, 1], f32, kind="ExternalInput")
    bq_in = nc.dram_tensor("bq", [C, 1], f32, kind="ExternalInput")
    bk_in = nc.dram_tensor("bk", [C, 1], f32, kind="ExternalInput")
    beff_in = nc.dram_tensor("beff", [C, 1], f32, kind="ExternalInput")
    wq_in = nc.dram_tensor("wqT", [C, C], f32, kind="ExternalInput")
    wk_in = nc.dram_tensor("wkT", [C, C], f32, kind="ExternalInput")
    wv_in = nc.dram_tensor("wvT", [C, C], f32, kind="ExternalInput")
    wp_in = nc.dram_tensor("wpT", [C, C], f32, kind="ExternalInput")
    ig_in = nc.dram_tensor("ig", [C, GROUPS], f32, kind="ExternalInput")
    igt_in = nc.dram_tensor("igt", [GROUPS, C], f32, kind="ExternalInput")
    out_dram = nc.dram_tensor("out", [C, HW], f32, kind="ExternalOutput")

    with tile.TileContext(nc) as tc, ExitStack() as ctx:
        const = ctx.enter_context(tc.tile_pool(name="const", bufs=1))
        big = ctx.enter_context(tc.tile_pool(name="big", bufs=1))
        stats = ctx.enter_context(tc.tile_pool(name="stats", bufs=1))
        ptpool = ctx.enter_context(tc.tile_pool(name="pt", bufs=24))
        tmpool = ctx.enter_context(tc.tile_pool(name="tmp", bufs=6))
        rrpool = ctx.enter_context(tc.tile_pool(name="rr", bufs=2))
        onrmp = ctx.enter_context(tc.tile_pool(name="onrm", bufs=3))
        ostg = ctx.enter_context(tc.tile_pool(name="ostg", bufs=3))
        ps = ctx.enter_context(tc.tile_pool(name="ps", bufs=1, space="PSUM"))

        # ---------------- load x (split over two DMA queues), consts ------
        NCH = 4
        CHW = HW // NCH  # 1024
        x_sb = big.tile([C, HW], f32, tag="x")
        xq = [nc.sync, nc.scalar, nc.gpsimd]
        for ch in range(8):
            sl = slice(ch * 512, (ch + 1) * 512)
            xq[ch % 3].dma_start(x_sb[:, sl], x_in[:, sl])

        def cload(t_in, shape, tag):
            t = const.tile(shape, f32, tag=tag)
            nc.sync.dma_start(t[:], t_in[:])
            return t

        gamma = cload(gamma_in, [C, 1], "c_gamma")
        beta = cload(beta_in, [C, 1], "c_beta")
        bq = cload(bq_in, [C, 1], "c_bq")
        bk = cload(bk_in, [C, 1], "c_bk")
        beff = cload(beff_in, [C, 1], "c_beff")
        ig = cload(ig_in, [C, GROUPS], "c_ig")
        igt = cload(igt_in, [GROUPS, C], "c_igt")
        wq_f = cload(wq_in, [C, C], "c_wq_f")
        wk_f = cload(wk_in, [C, C], "c_wk_f")
        wv_f = cload(wv_in, [C, C], "c_wv_f")
        wp_f = cload(wp_in, [C, C], "c_wp_f")

        with nc.allow_low_precision(reason="bf16 weights: rel tol is 2e-2"):
            wq_b = const.tile([C, C], bf16)
            nc.vector.tensor_copy(wq_b[:], wq_f[:])
            wk_b = const.tile([C, C], bf16)
            nc.vector.tensor_copy(wk_b[:], wk_f[:])
            wv_b = const.tile([C, C], bf16)
            nc.vector.tensor_copy(wv_b[:], wv_f[:])
            wp_b = const.tile([C, C], bf16)
            nc.vector.tensor_copy(wp_b[:], wp_f[:])

        ones_c = const.tile([C, 1], bf16)
        nc.vector.memset(ones_c[:], 1.0)
        ones_r = const.tile([1, C], bf16)
        nc.vector.memset(ones_r[:], 1.0)
        eps_t = const.tile([GROUPS, 1], f32)
        nc.vector.memset(eps_t[:], EPS)
        magic_t = const.tile([GROUPS, 1], mybir.dt.uint32)
        nc.vector.memset(magic_t[:], 0x5F3759DF)
        c15_t = const.tile([GROUPS, 1], f32)
        nc.vector.memset(c15_t[:], 1.5)

        # ---------------- groupnorm stats via bn_stats ----------------
        bnst = stats.tile([C, 8, 6], f32)
        for ch in range(8):
            sl = slice(ch * 512, (ch + 1) * 512)
            nc.vector.bn_stats(bnst[:, ch, :], x_sb[:, sl])
        mv = stats.tile([C, 2], f32)  # per-channel mean, var
        nc.vector.bn_aggr(mv[:], bnst[:])
        # warm the exp activation table before the loop
        warm = stats.tile([GROUPS, 1], f32)
        nc.scalar.activation(warm[:], eps_t[:], AF.Exp)

        # pack [mean, var + mean^2] -> group sums via indicator matmul
        msq = stats.tile([C, 2], f32)
        nc.vector.tensor_copy(msq[:, 0:1], mv[:, 0:1])
        nc.vector.tensor_mul(msq[:, 1:2], mv[:, 0:1], mv[:, 0:1])
        nc.vector.tensor_add(msq[:, 1:2], msq[:, 1:2], mv[:, 1:2])

        # ig is host-scaled by 1/GSIZE so the matmul yields means directly
        gs_ps = ps.tile([GROUPS, 2], f32, tag="pp0")
        nc.tensor.matmul(gs_ps[:], ig[:], msq[:], start=True, stop=True)
        gmr = stats.tile([GROUPS, 2], f32)
        nc.vector.tensor_copy(gmr[:, 0:1], gs_ps[:, 0:1])
        gmsq = stats.tile([GROUPS, 1], f32)
        nc.vector.tensor_mul(gmsq[:], gmr[:, 0:1], gmr[:, 0:1])
        gve = stats.tile([GROUPS, 1], f32)
        nc.vector.tensor_sub(gve[:], gs_ps[:, 1:2], gmsq[:])
        nc.vector.tensor_scalar(
            gve[:], gve[:], eps_t[:], None, mybir.AluOpType.add
        )
        # rstd = rsqrt(var+eps): quake guess + 1 Newton step (rel err
        # ~1.7e-3, far below the bf16 data-path noise)
        u32 = mybir.dt.uint32
        gu = stats.tile([GROUPS, 1], u32)
        nc.vector.tensor_scalar(
            gu[:], gve[:].bitcast(u32), 1, None,
            mybir.AluOpType.logical_shift_right,
        )
        nc.vector.tensor_sub(gu[:], magic_t[:], gu[:])
        gy = stats.tile([GROUPS, 1], f32)
        nc.vector.tensor_copy(gy[:], gu[:].bitcast(f32))
        gh = stats.tile([GROUPS, 1], f32)
        nc.vector.tensor_scalar_mul(gh[:], gve[:], 0.5)
        gt = stats.tile([GROUPS, 1], f32)
        nc.vector.tensor_mul(gt[:], gy[:], gy[:])
        nc.vector.tensor_mul(gt[:], gt[:], gh[:])
        nc.vector.tensor_sub(gt[:], c15_t[:], gt[:])
        nc.vector.tensor_mul(gmr[:, 1:2], gy[:], gt[:])

        bc_ps = ps.tile([C, 2], f32, tag="pp1")
        nc.tensor.matmul(bc_ps[:], igt[:], gmr[:], start=True, stop=True)
        a_c = stats.tile([C, 1], f32)
        b_c = stats.tile([C, 1], f32)
        tmc = stats.tile([C, 1], f32)
        nc.vector.tensor_scalar_mul(a_c[:], gamma[:], bc_ps[:, 1:2])
        nc.vector.tensor_scalar_mul(tmc[:], a_c[:], bc_ps[:, 0:1])
        nc.vector.tensor_sub(b_c[:], beta[:], tmc[:])

        # ---------------- hn (bf16) and QKV ----------------
        hn = big.tile([C, HW], bf16, tag="hn")
        q_r = big.tile([C, HW], bf16, tag="q")
        k_r = big.tile([C, HW], bf16, tag="k")
        vt = big.tile([C, NJ, 128], bf16, tag="vt")
        acc = big.tile([C, HW], bf16, tag="acc")
        o_sb = big.tile([C, HW], bf16, tag="o")

        lp = nc.allow_low_precision(reason="bf16 data path: rel tol is 2e-2")
        lp.__enter__()

        def emit_hn(ch):
            sl = slice(ch * CHW, (ch + 1) * CHW)
            nc.vector.tensor_scalar(
                hn[:, sl], x_sb[:, sl], a_c[:], b_c[:],
                mybir.AluOpType.mult, mybir.AluOpType.add,
            )

        emit_hn(0)  # chunk 0 unblocks K0/Q0/V0; the rest follow

        def emit_k_half(r, h):  # [C, 512] half on a pp bank
            kp = ps.tile([C, 512], f32, tag=f"pp{h}")
            off = r * QW + h * 512
            nc.tensor.matmul(
                kp[:], wk_b[:], hn[:, off:off + 512], start=True, stop=True
            )
            nc.scalar.activation(
                k_r[:, off:off + 512], kp[:], AF.Identity, bias=bk[:]
            )

        def emit_q_half(r, h, on_act=False):
            qp = ps.tile([C, 512], f32, tag=f"pp{h}")
            off = r * QW + h * 512
            nc.tensor.matmul(
                qp[:], wq_b[:], hn[:, off:off + 512], start=True, stop=True
            )
            if on_act:  # preamble only: ACT is idle there, DVE is not
                nc.scalar.activation(
                    q_r[:, off:off + 512], qp[:], AF.Identity, bias=bq[:]
                )
            else:
                nc.vector.tensor_scalar(
                    q_r[:, off:off + 512], qp[:], bq[:], None,
                    mybir.AluOpType.add,
                )

        def emit_v_round(r):  # 4 n-tiles per round, on a pp bank
            vp = ps.tile([C, 4, C], f32, tag=f"pp{r % 2}")
            for t in range(4):
                nt = r * 4 + t
                nc.tensor.matmul(
                    vp[:, t, :], hn[:, nt * 128:(nt + 1) * 128], wv_b[:],
                    start=True, stop=True,
                )
            nc.vector.tensor_copy(vt[:, r * 4:(r + 1) * 4, :], vp[:])

        v_nat = big.tile([C, HW], bf16, tag="vnat")
        vtq = [nc.sync, nc.scalar]

        def emit_v_chunk(ch):
            # V rounds 1-7 via one natural-layout matmul + xbar DMA
            # transposes on idle queues: 1 PE matmul instead of 4
            vnp = ps.tile([C, 512], f32, tag=f"pp{ch % 2}")
            sl = slice(ch * 512, (ch + 1) * 512)
            nc.tensor.matmul(vnp[:], wv_b[:], hn[:, sl], start=True, stop=True)
            nc.vector.tensor_copy(v_nat[:, sl], vnp[:])
            for t in range(4):
                nt = ch * 4 + t
                vtq[nt % 2].dma_start_transpose(
                    vt[:, nt, :], v_nat[:, nt * 128:(nt + 1) * 128]
                )

        # K/Q for the first 1024 queries/keys + the first V tiles before
        # the loop (biases on ACT/DVE: the Pool queue would serialize the
        # critical path here); the rest of K, V, and Q are interleaved
        # into block 0's j-loop with biases on the idle Pool engine
        emit_k_half(0, 0)
        emit_k_half(0, 1)
        emit_q_half(0, 0, on_act=True)
        emit_q_half(0, 1, on_act=True)
        emit_v_round(0)
        for ch in range(1, NCH):
            emit_hn(ch)

        # ---------------- main attention loop ----------------
        # The softmax-denominator + projection chain for block qt-1 runs
        # entirely on PE/DVE/DMA, time-multiplexing the two pp PSUM banks
        # (rowsum -> broadcast -> projection), with stages pinned to well-
        # separated emission points in block qt's j-loop so no engine queue
        # ever blocks on a pending cross-engine chain.
        chain = {}

        def den_rowsum(qt, c2):
            sl = slice(qt * QW + c2 * 512, qt * QW + (c2 + 1) * 512)
            dps = ps.tile([1, 512], f32, tag=f"pp{c2}")
            nc.tensor.matmul(dps[:], ones_c[:], acc[:, sl], start=True,
                             stop=True)
            den_row = rrpool.tile([1, 512], f32, tag=f"drow{c2}")
            nc.vector.tensor_copy(den_row[:], dps[:])
            rs = rrpool.tile([C, 4], f32, tag=f"rs{c2}")
            nc.sync.dma_start(rs[:], den_row[:])
            chain[f"rs{c2}"] = rs

        def den_recip(c2):
            rc = rrpool.tile([C, 4], bf16, tag=f"rc{c2}")
            nc.vector.reciprocal(rc[:], chain[f"rs{c2}"][:])
            rrow = rrpool.tile([1, 512], bf16, tag=f"rrow{c2}")
            nc.sync.dma_start(rrow[:], rc[:])
            chain[f"rrow{c2}"] = rrow

        def den_bcast(c2):
            bps = ps.tile([C, 512], f32, tag=f"pp{c2}")
            nc.tensor.matmul(bps[:], ones_r[:], chain[f"rrow{c2}"][:],
                             start=True, stop=True)
            chain[f"bps{c2}"] = bps

        def proj_onrm(qt, c2):
            sl = slice(qt * QW + c2 * 512, qt * QW + (c2 + 1) * 512)
            onrm = onrmp.tile([C, 512], bf16)
            nc.vector.tensor_mul(onrm[:], o_sb[:, sl], chain[f"bps{c2}"][:])
            chain[f"onrm{c2}"] = onrm

        def proj_store(qt, c2):
            sl = slice(qt * QW + c2 * 512, qt * QW + (c2 + 1) * 512)
            pp = ps.tile([C, 512], f32, tag=f"pp{c2}")
            nc.tensor.matmul(pp[:], wp_b[:], chain[f"onrm{c2}"][:],
                             start=True, stop=True)
            ost = ostg.tile([C, 512], f32)
            nc.vector.tensor_add(ost[:], pp[:], x_sb[:, sl])
            nc.sync.dma_start(out_dram[:, sl], ost[:])

        for qt in range(NQT):
            qsl = slice(qt * QW, (qt + 1) * QW)
            op0 = ps.tile([C, 512], f32, tag="o0")
            op1 = ps.tile([C, 512], f32, tag="o1")
            ops = [op0, op1]
            pts = [None, None]
            for j in range(NJ):
                if qt == 0:
                    if j % 4 == 0 and j > 0:
                        emit_v_round(j // 4)
                    if j % 8 == 6 and j < 24:
                        r = j // 8 + 1
                        emit_k_half(r, 0)
                        emit_k_half(r, 1)
                if qt == 0 and j in (2, 4):
                    # residual+bias prefold: x_sb <- x + beff (x no longer
                    # needed raw; first consumer is proj(q0) at qt1 j24)
                    sl2 = slice((j // 2 - 1) * 2048, (j // 2) * 2048)
                    nc.vector.tensor_scalar(
                        x_sb[:, sl2], x_sb[:, sl2], beff[:], None,
                        mybir.AluOpType.add,
                    )
                if qt > 0:
                    if j == 3:
                        den_rowsum(qt - 1, 0)
                    elif j == 4:
                        den_rowsum(qt - 1, 1)
                    elif j == 7:
                        den_recip(0)
                    elif j == 8:
                        den_recip(1)
                    elif j == 11:
                        den_bcast(0)
                    elif j == 12:
                        den_bcast(1)
                    elif j == 13:
                        proj_onrm(qt - 1, 0)
                    elif j == 14:
                        proj_onrm(qt - 1, 1)
                    elif j == 15:
                        proj_store(qt - 1, 0)
                    elif j == 16:
                        proj_store(qt - 1, 1)
                if j == 17 and qt < NQT - 1:
                    emit_q_half(qt + 1, 0)
                if j == 19 and qt < NQT - 1:
                    emit_q_half(qt + 1, 1)
                sp = ps.tile([C, QW], f32, tag=f"s{j % 2}")
                for kk in range(2):
                    qoff = qt * QW + kk * 512
                    nc.tensor.matmul(
                        sp[:, kk * 512:(kk + 1) * 512],
                        k_r[:, j * 128:(j + 1) * 128],
                        q_r[:, qoff:qoff + 512],
                        start=True, stop=True,
                    )
                pt = ptpool.tile([C, QW], bf16)
                nc.scalar.activation(pt[:], sp[:], AF.Exp)
                for kk in range(2):
                    sl = slice(kk * 512, (kk + 1) * 512)
                    nc.tensor.matmul(
                        ops[kk][:], vt[:, j, :], pt[:, sl],
                        start=(j == 0), stop=(j == NJ - 1),
                    )
                pts[j % 2] = pt
                if j % 2 == 1:
                    if j == 1:
                        nc.vector.tensor_add(acc[:, qsl], pts[0][:], pts[1][:])
                    else:
                        tmp = tmpool.tile([C, QW], bf16)
                        nc.vector.tensor_add(tmp[:], pts[0][:], pts[1][:])
                        nc.vector.tensor_add(acc[:, qsl], acc[:, qsl], tmp[:])

            # ---- block epilogue ----
            # o_sb copy on ACT: it releases the o_ps PSUM slot that gates
            # the next block's first O matmul, and must not queue behind
            # the DVE's accumulator backlog
            for kk in range(2):
                osl = slice(qt * QW + kk * 512, qt * QW + (kk + 1) * 512)
                nc.scalar.activation(o_sb[:, osl], ops[kk][:], AF.Copy)
            if qt == NQT - 1:
                # final block: PSUM s-slots are free now -- PE rowsum and
                # broadcast matmuls avoid the slow serial gpsimd chain.
                # Pipelined per 512-chunk to shorten the exposed tail.
                for c2 in range(2):
                    off = qt * QW + c2 * 512
                    sl = slice(off, off + 512)
                    dps = ps.tile([1, 512], f32, tag="s0")
                    nc.tensor.matmul(
                        dps[:], ones_c[:], acc[:, sl], start=True, stop=True
                    )
                    den_row = rrpool.tile([1, 512], f32, tag="drow")
                    nc.vector.tensor_copy(den_row[:], dps[:])
                    rs = rrpool.tile([C, 4], f32, tag="rs")
                    nc.sync.dma_start(rs[:], den_row[:])
                    rc = rrpool.tile([C, 4], bf16, tag="rc")
                    nc.vector.reciprocal(rc[:], rs[:])
                    rrow = rrpool.tile([1, 512], bf16, tag="rrow")
                    nc.sync.dma_start(rrow[:], rc[:])
                    bps = ps.tile([C, 512], f32, tag="s1")
                    nc.tensor.matmul(
                        bps[:], ones_r[:], rrow[:], start=True, stop=True
                    )
                    onrm = onrmp.tile([C, 512], bf16)
                    nc.vector.tensor_mul(onrm[:], o_sb[:, sl], bps[:])
                    pp = ps.tile([C, 512], f32, tag=f"pp{c2}")
                    nc.tensor.matmul(
                        pp[:], wp_b[:], onrm[:], start=True, stop=True
                    )
                    ost = ostg.tile([C, 512], f32)
                    nc.vector.tensor_add(ost[:], pp[:], x_sb[:, sl])
                    nc.sync.dma_start(out_dram[:, sl], ost[:])

        lp.__exit__(None, None, None)

    nc.compile()
    return nc


def _get_nc():
    if "nc" not in _CACHE:
        _CACHE["nc"] = _build()
    return _CACHE["nc"]


def _prep_inputs(x, gamma, beta, w_qkv, b_qkv, w_proj, b_proj):
    x = np.ascontiguousarray(x, dtype=np.float32)
    w_qkv = np.asarray(w_qkv, dtype=np.float32)
    b_qkv = np.asarray(b_qkv, dtype=np.float32)
    w_proj = np.asarray(w_proj, dtype=np.float32)
    b_proj = np.asarray(b_proj, dtype=np.float32)

    wq = w_qkv[0:C, :]
    wk = w_qkv[C:2 * C, :]
    wv = w_qkv[2 * C:3 * C, :]
    bqv = b_qkv[0:C]
    bkv = b_qkv[C:2 * C]
    bvv = b_qkv[2 * C:3 * C]

    wqT = np.ascontiguousarray((wq * SCALE).T)
    wkT = np.ascontiguousarray(wk.T)
    wvT = np.ascontiguousarray(wv.T)
    wpT = np.ascontiguousarray(w_proj.T)
    beff = (b_proj + w_proj @ bvv).astype(np.float32)

    ig = np.zeros((C, GROUPS), np.float32)
    ig[np.arange(C), np.arange(C) // GSIZE] = 1.0
    igt = np.ascontiguousarray(ig.T)
    ig = ig * (1.0 / GSIZE)  # fold the group-mean divide into the matmul

    common = {
        "gamma": np.asarray(gamma, np.float32).reshape(C, 1),
        "beta": np.asarray(beta, np.float32).reshape(C, 1),
        "bq": (bqv * SCALE).reshape(C, 1),
        "bk": bkv.reshape(C, 1),
        "beff": beff.reshape(C, 1),
        "wqT": wqT,
        "wkT": wkT,
        "wvT": wvT,
        "wpT": wpT,
        "ig": ig,
        "igt": igt,
    }
    in_maps = []
    for b in range(B):
        m = dict(common)
        m["x"] = np.ascontiguousarray(x[b].reshape(C, HW))
        in_maps.append(m)
    return in_maps


def kernel(x, gamma, beta, w_qkv, b_qkv, w_proj, b_proj):
    from concourse.bass_utils import run_bass_kernel_spmd

    nc = _get_nc()
    in_maps = _prep_inputs(x, gamma, beta, w_qkv, b_qkv, w_proj, b_proj)
    res = run_bass_kernel_spmd(nc, in_maps, list(range(B)))
    out = np.stack([res.results[b]["out"] for b in range(B)], axis=0)
    return out.reshape(B, C, H, W).astype(np.float32)


# revision 51
# speedup vs baseline: 1.0101x; 1.0101x over previous
"""Trainium2 Bass kernel for nn_AttentionBlock (GroupNorm + 1x1-conv QKV +
full self-attention over N=HW=4096 + output projection + residual).

Distribution: data-parallel over batch B=8, one batch element per NeuronCore.

v2: all matmul operands in bf16 (fp32 matmuls stream at ~2 cyc/col on TRN2
hardware -- "fp32_mode=HIGH/LOW" dual pass -- so bf16 doubles PE throughput).
The loop processes queries in 4 blocks of 1024 so exp runs as a single
[128,1024] ACT instruction per j-tile, with double-buffered S tiles in PSUM:

  per (block, j):  S^T = K_j^T Q_blk   (PE, 2x 512-col matmuls, bf16)
                   P^T = exp(S^T)      (ACT, PSUM->SBUF bf16)
                   O  += V_j^T P^T     (PE, 2x 512-col, PSUM accum over j)
                   acc += P^T          (DVE, bf16 pair tree)

Softmax denominators: bf16 pair-tree accumulation on DVE, cross-partition
sum via gpsimd partition_all_reduce (idle Pool engine), reciprocal on a
DMA-scattered [128,8] layout, broadcast back via gpsimd partition_broadcast.
Tail uses proj/normalize commutation: O is normalized per-query BEFORE the
projection (they commute since the denominator is a per-query scalar), so
out = (w_proj^T (O * recip)) + (x + b_eff) with the residual pre-folded.

Bias algebra: b_k folded into the K PSUM->SBUF copies (ACT bias); b_q into
the Q copies (DVE); b_v folded into b_eff = b_proj + w_proj @ b_v (host,
exact). The attention scale C^-0.5 is folded into w_q/b_q on the host.
No max-subtraction in softmax: logits are ~N(0,1) so fp32 exp is safe.
"""

import numpy as np

B, C, H, W = 8, 128, 64, 64
HW = H * W                      # 4096
GROUPS = 8
GSIZE = C // GROUPS             # 16
EPS = 1e-5
NJ = HW // 128                  # 32 j-tiles
QW = 1024                       # queries per block
NQT = HW // QW                  # 4 blocks
SCALE = float(C) ** -0.5

_CACHE = {}


def _build():
    from contextlib import ExitStack

    import concourse.bacc as bacc
    import concourse.tile as tile
    from concourse import bass_isa, mybir

    f32 = mybir.dt.float32
    bf16 = mybir.dt.bfloat16
    AF = mybir.ActivationFunctionType

    nc = bacc.Bacc("TRN2", target_bir_lowering=False, debug=False)

    x_in = nc.dram_tensor("x", [C, HW], f32, kind="ExternalInput")
    gamma_in = nc.dram_tensor("gamma", [C, 1], f32, kind="ExternalInput")
    beta_in = nc.dram_tensor("beta", [C, 1], f32, kind="ExternalInput")
    bq_in = nc.dram_tensor("bq", [C, 1], f32, kind="ExternalInput")
    bk_in = nc.dram_tensor("bk", [C, 1], f32, kind="ExternalInput")
    beff_in = nc.dram_tensor("beff", [C, 1], f32, kind="ExternalInput")
    wq_in = nc.dram_tensor("wqT", [C, C], f32, kind="ExternalInput")
    wk_in = nc.dram_tensor("wkT", [C, C], f32, kind="ExternalInput")
    wv_in = nc.dram_tensor("wvT", [C, C], f32, kind="ExternalInput")
    wp_in = nc.dram_tensor("wpT", [C, C], f32, kind="ExternalInput")
    ig_in = nc.dram_tensor("ig", [C, GROUPS], f32, kind="ExternalInput")
    igt_in = nc.dram_tensor("igt", [GROUPS, C], f32, kind="ExternalInput")
    out_dram = nc.dram_tensor("out", [C, HW], f32, kind="ExternalOutput")

    with tile.TileContext(nc) as tc, ExitStack() as ctx:
        const = ctx.enter_context(tc.tile_pool(name="const", bufs=1))
        big = ctx.enter_context(tc.tile_pool(name="big", bufs=1))
        stats = ctx.enter_context(tc.tile_pool(name="stats", bufs=1))
        ptpool = ctx.enter_context(tc.tile_pool(name="pt", bufs=24))
        tmpool = ctx.enter_context(tc.tile_pool(name="tmp", bufs=6))
        rrpool = ctx.enter_context(tc.tile_pool(name="rr", bufs=2))
        onrmp = ctx.enter_context(tc.tile_pool(name="onrm", bufs=3))
        ostg = ctx.enter_context(tc.tile_pool(name="ostg", bufs=3))
        ps = ctx.enter_context(tc.tile_pool(name="ps", bufs=1, space="PSUM"))

        # ---------------- load x (split over two DMA queues), consts ------
        NCH = 4
        CHW = HW // NCH  # 1024
        x_sb = big.tile([C, HW], f32, tag="x")
        xq = [nc.sync, nc.scalar, nc.gpsimd]
        for ch in range(8):
            sl = slice(ch * 512, (ch + 1) * 512)
            xq[ch % 3].dma_start(x_sb[:, sl], x_in[:, sl])

        def cload(t_in, shape, tag):
            t = const.tile(shape, f32, tag=tag)
            nc.sync.dma_start(t[:], t_in[:])
            return t

        gamma = cload(gamma_in, [C, 1], "c_gamma")
        beta = cload(beta_in, [C, 1], "c_beta")
        bq = cload(bq_in, [C, 1], "c_bq")
        bk = cload(bk_in, [C, 1], "c_bk")
        beff = cload(beff_in, [C, 1], "c_beff")
        ig = cload(ig_in, [C, GROUPS], "c_ig")
        igt = cload(igt_in, [GROUPS, C], "c_igt")
        wq_f = cload(wq_in, [C, C], "c_wq_f")
        wk_f = cload(wk_in, [C, C], "c_wk_f")
        wv_f = cload(wv_in, [C, C], "c_wv_f")
        wp_f = cload(wp_in, [C, C], "c_wp_f")

        with nc.allow_low_precision(reason="bf16 weights: rel tol is 2e-2"):
            wq_b = const.tile([C, C], bf16)
            nc.vector.tensor_copy(wq_b[:], wq_f[:])
            wk_b = const.tile([C, C], bf16)
            nc.vector.tensor_copy(wk_b[:], wk_f[:])
            wv_b = const.tile([C, C], bf16)
            nc.vector.tensor_copy(wv_b[:], wv_f[:])
            wp_b = const.tile([C, C], bf16)
            nc.vector.tensor_copy(wp_b[:], wp_f[:])

        ones_c = const.tile([C, 1], bf16)
        nc.vector.memset(ones_c[:], 1.0)
        ones_r = const.tile([1, C], bf16)
        nc.vector.memset(ones_r[:], 1.0)
        eps_t = const.tile([GROUPS, 1], f32)
        nc.vector.memset(eps_t[:], EPS)
        magic_t = const.tile([GROUPS, 1], mybir.dt.uint32)
        nc.vector.memset(magic_t[:], 0x5F3759DF)
        c15_t = const.tile([GROUPS, 1], f32)
        nc.vector.memset(c15_t[:], 1.5)

        # ---------------- groupnorm stats via bn_stats ----------------
        bnst = stats.tile([C, 8, 6], f32)
        for ch in range(8):
            sl = slice(ch * 512, (ch + 1) * 512)
            nc.vector.bn_stats(bnst[:, ch, :], x_sb[:, sl])
        mv = stats.tile([C, 2], f32)  # per-channel mean, var
        nc.vector.bn_aggr(mv[:], bnst[:])
        # warm the exp activation table before the loop
        warm = stats.tile([GROUPS, 1], f32)
        nc.scalar.activation(warm[:], eps_t[:], AF.Exp)

        # pack [mean, var + mean^2] -> group sums via indicator matmul
        msq = stats.tile([C, 2], f32)
        nc.vector.tensor_copy(msq[:, 0:1], mv[:, 0:1])
        nc.vector.tensor_mul(msq[:, 1:2], mv[:, 0:1], mv[:, 0:1])
        nc.vector.tensor_add(msq[:, 1:2], msq[:, 1:2], mv[:, 1:2])

        # ig is host-scaled by 1/GSIZE so the matmul yields means directly
        gs_ps = ps.tile([GROUPS, 2], f32, tag="pp0")
        nc.tensor.matmul(gs_ps[:], ig[:], msq[:], start=True, stop=True)
        gmr = stats.tile([GROUPS, 2], f32)
        nc.vector.tensor_copy(gmr[:, 0:1], gs_ps[:, 0:1])
        gmsq = stats.tile([GROUPS, 1], f32)
        nc.vector.tensor_mul(gmsq[:], gmr[:, 0:1], gmr[:, 0:1])
        gve = stats.tile([GROUPS, 1], f32)
        nc.vector.tensor_sub(gve[:], gs_ps[:, 1:2], gmsq[:])
        nc.vector.tensor_scalar(
            gve[:], gve[:], eps_t[:], None, mybir.AluOpType.add
        )
        # rstd = rsqrt(var+eps): quake guess + 1 Newton step (rel err
        # ~1.7e-3, far below the bf16 data-path noise)
        u32 = mybir.dt.uint32
        gu = stats.tile([GROUPS, 1], u32)
        nc.vector.tensor_scalar(
            gu[:], gve[:].bitcast(u32), 1, None,
            mybir.AluOpType.logical_shift_right,
        )
        nc.vector.tensor_sub(gu[:], magic_t[:], gu[:])
        gy = stats.tile([GROUPS, 1], f32)
        nc.vector.tensor_copy(gy[:], gu[:].bitcast(f32))
        gh = stats.tile([GROUPS, 1], f32)
        nc.vector.tensor_scalar_mul(gh[:], gve[:], 0.5)
        gt = stats.tile([GROUPS, 1], f32)
        nc.vector.tensor_mul(gt[:], gy[:], gy[:])
        nc.vector.tensor_mul(gt[:], gt[:], gh[:])
        nc.vector.tensor_sub(gt[:], c15_t[:], gt[:])
        nc.vector.tensor_mul(gmr[:, 1:2], gy[:], gt[:])

        bc_ps = ps.tile([C, 2], f32, tag="pp1")
        nc.tensor.matmul(bc_ps[:], igt[:], gmr[:], start=True, stop=True)
        a_c = stats.tile([C, 1], f32)
        b_c = stats.tile([C, 1], f32)
        tmc = stats.tile([C, 1], f32)
        nc.vector.tensor_scalar_mul(a_c[:], gamma[:], bc_ps[:, 1:2])
        nc.vector.tensor_scalar_mul(tmc[:], a_c[:], bc_ps[:, 0:1])
        nc.vector.tensor_sub(b_c[:], beta[:], tmc[:])

        # ---------------- hn (bf16) and QKV ----------------
        hn = big.tile([C, HW], bf16, tag="hn")
        q_r = big.tile([C, HW], bf16, tag="q")
        k_r = big.tile([C, HW], bf16, tag="k")
        vt = big.tile([C, NJ, 128], bf16, tag="vt")
        acc = big.tile([C, HW], bf16, tag="acc")
        o_sb = big.tile([C, HW], bf16, tag="o")

        lp = nc.allow_low_precision(reason="bf16 data path: rel tol is 2e-2")
        lp.__enter__()

        def emit_hn(ch):
            sl = slice(ch * CHW, (ch + 1) * CHW)
            nc.vector.tensor_scalar(
                hn[:, sl], x_sb[:, sl], a_c[:], b_c[:],
                mybir.AluOpType.mult, mybir.AluOpType.add,
            )

        emit_hn(0)  # chunk 0 unblocks K0/Q0/V0; the rest follow

        def emit_k_half(r, h):  # [C, 512] half on a pp bank
            kp = ps.tile([C, 512], f32, tag=f"pp{h}")
            off = r * QW + h * 512
            nc.tensor.matmul(
                kp[:], wk_b[:], hn[:, off:off + 512], start=True, stop=True
            )
            nc.scalar.activation(
                k_r[:, off:off + 512], kp[:], AF.Identity, bias=bk[:]
            )

        def emit_q_half(r, h, on_act=False):
            qp = ps.tile([C, 512], f32, tag=f"pp{h}")
            off = r * QW + h * 512
            nc.tensor.matmul(
                qp[:], wq_b[:], hn[:, off:off + 512], start=True, stop=True
            )
            if on_act:  # preamble only: ACT is idle there, DVE is not
                nc.scalar.activation(
                    q_r[:, off:off + 512], qp[:], AF.Identity, bias=bq[:]
                )
            else:
                nc.vector.tensor_scalar(
                    q_r[:, off:off + 512], qp[:], bq[:], None,
                    mybir.AluOpType.add,
                )

        def emit_v_round(r):  # 4 n-tiles per round, on a pp bank
            vp = ps.tile([C, 4, C], f32, tag=f"pp{r % 2}")
            for t in range(4):
                nt = r * 4 + t
                nc.tensor.matmul(
                    vp[:, t, :], hn[:, nt * 128:(nt + 1) * 128], wv_b[:],
                    start=True, stop=True,
                )
            nc.vector.tensor_copy(vt[:, r * 4:(r + 1) * 4, :], vp[:])

        v_nat = big.tile([C, HW], bf16, tag="vnat")
        vtq = [nc.sync, nc.scalar]

        def emit_v_chunk(ch):
            # V rounds 1-7 via one natural-layout matmul + xbar DMA
            # transposes on idle queues: 1 PE matmul instead of 4
            vnp = ps.tile([C, 512], f32, tag=f"pp{ch % 2}")
            sl = slice(ch * 512, (ch + 1) * 512)
            nc.tensor.matmul(vnp[:], wv_b[:], hn[:, sl], start=True, stop=True)
            nc.vector.tensor_copy(v_nat[:, sl], vnp[:])
            for t in range(4):
                nt = ch * 4 + t
                vtq[nt % 2].dma_start_transpose(
                    vt[:, nt, :], v_nat[:, nt * 128:(nt + 1) * 128]
                )

        # K/Q for the first 1024 queries/keys + the first V tiles before
        # the loop (biases on ACT/DVE: the Pool queue would serialize the
        # critical path here); the rest of K, V, and Q are interleaved
        # into block 0's j-loop with biases on the idle Pool engine
        emit_k_half(0, 0)
        emit_k_half(0, 1)
        emit_q_half(0, 0, on_act=True)
        emit_q_half(0, 1, on_act=True)
        emit_v_round(0)
        for ch in range(1, NCH):
            emit_hn(ch)

        # ---------------- main attention loop ----------------
        # The softmax-denominator + projection chain for block qt-1 runs
        # entirely on PE/DVE/DMA, time-multiplexing the two pp PSUM banks
        # (rowsum -> broadcast -> projection), with stages pinned to well-
        # separated emission points in block qt's j-loop so no engine queue
        # ever blocks on a pending cross-engine chain.
        chain = {}

        def den_rowsum(qt, c2):
            sl = slice(qt * QW + c2 * 512, qt * QW + (c2 + 1) * 512)
            dps = ps.tile([1, 512], f32, tag=f"pp{c2}")
            nc.tensor.matmul(dps[:], ones_c[:], acc[:, sl], start=True,
                             stop=True)
            den_row = rrpool.tile([1, 512], f32, tag=f"drow{c2}")
            nc.vector.tensor_copy(den_row[:], dps[:])
            rs = rrpool.tile([C, 4], f32, tag=f"rs{c2}")
            nc.sync.dma_start(rs[:], den_row[:])
            chain[f"rs{c2}"] = rs

        def den_recip(c2):
            rc = rrpool.tile([C, 4], bf16, tag=f"rc{c2}")
            nc.vector.reciprocal(rc[:], chain[f"rs{c2}"][:])
            rrow = rrpool.tile([1, 512], bf16, tag=f"rrow{c2}")
            nc.sync.dma_start(rrow[:], rc[:])
            chain[f"rrow{c2}"] = rrow

        def den_bcast(c2):
            bps = ps.tile([C, 512], f32, tag=f"pp{c2}")
            nc.tensor.matmul(bps[:], ones_r[:], chain[f"rrow{c2}"][:],
                             start=True, stop=True)
            chain[f"bps{c2}"] = bps

        def proj_onrm(qt, c2):
            sl = slice(qt * QW + c2 * 512, qt * QW + (c2 + 1) * 512)
            onrm = onrmp.tile([C, 512], bf16)
            nc.vector.tensor_mul(onrm[:], o_sb[:, sl], chain[f"bps{c2}"][:])
            chain[f"onrm{c2}"] = onrm

        def proj_store(qt, c2):
            sl = slice(qt * QW + c2 * 512, qt * QW + (c2 + 1) * 512)
            pp = ps.tile([C, 512], f32, tag=f"pp{c2}")
            nc.tensor.matmul(pp[:], wp_b[:], chain[f"onrm{c2}"][:],
                             start=True, stop=True)
            ost = ostg.tile([C, 512], f32)
            nc.vector.tensor_add(ost[:], pp[:], x_sb[:, sl])
            nc.sync.dma_start(out_dram[:, sl], ost[:])

        for qt in range(NQT):
            qsl = slice(qt * QW, (qt + 1) * QW)
            op0 = ps.tile([C, 512], f32, tag="o0")
            op1 = ps.tile([C, 512], f32, tag="o1")
            ops = [op0, op1]
            pts = [None, None]
            for j in range(NJ):
                if qt == 0:
                    if j % 4 == 0 and j > 0:
                        emit_v_round(j // 4)
                    if j % 8 == 6 and j < 24:
                        r = j // 8 + 1
                        emit_k_half(r, 0)
                        emit_k_half(r, 1)
                if qt in (1, 2) and j == 5:
                    # residual+bias prefold: x_sb <- x + beff, placed in the
                    # DVE-slack quarters (chunk qt-1 feeds ost at j15)
                    sl2 = slice((qt - 1) * 2048, qt * 2048)
                    nc.vector.tensor_scalar(
                        x_sb[:, sl2], x_sb[:, sl2], beff[:], None,
                        mybir.AluOpType.add,
                    )
                if qt > 0:
                    if j == 3:
                        den_rowsum(qt - 1, 0)
                    elif j == 4:
                        den_rowsum(qt - 1, 1)
                    elif j == 7:
                        den_recip(0)
                    elif j == 8:
                        den_recip(1)
                    elif j == 11:
                        den_bcast(0)
                    elif j == 12:
                        den_bcast(1)
                    elif j == 13:
                        proj_onrm(qt - 1, 0)
                    elif j == 14:
                        proj_onrm(qt - 1, 1)
                    elif j == 15:
                        proj_store(qt - 1, 0)
                    elif j == 16:
                        proj_store(qt - 1, 1)
                if j == 17 and qt < NQT - 1:
                    emit_q_half(qt + 1, 0)
                if j == 19 and qt < NQT - 1:
                    emit_q_half(qt + 1, 1)
                sp = ps.tile([C, QW], f32, tag=f"s{j % 2}")
                for kk in range(2):
                    qoff = qt * QW + kk * 512
                    nc.tensor.matmul(
                        sp[:, kk * 512:(kk + 1) * 512],
                        k_r[:, j * 128:(j + 1) * 128],
                        q_r[:, qoff:qoff + 512],
                        start=True, stop=True,
                    )
                pt = ptpool.tile([C, QW], bf16)
                nc.scalar.activation(pt[:], sp[:], AF.Exp)
                for kk in range(2):
                    sl = slice(kk * 512, (kk + 1) * 512)
                    nc.tensor.matmul(
                        ops[kk][:], vt[:, j, :], pt[:, sl],
                        start=(j == 0), stop=(j == NJ - 1),
                    )
                pts[j % 2] = pt
                if j % 2 == 1:
                    if j == 1:
                        nc.vector.tensor_add(acc[:, qsl], pts[0][:], pts[1][:])
                    else:
                        tmp = tmpool.tile([C, QW], bf16)
                        nc.vector.tensor_add(tmp[:], pts[0][:], pts[1][:])
                        nc.vector.tensor_add(acc[:, qsl], acc[:, qsl], tmp[:])

            # ---- block epilogue ----
            # o_sb copy on ACT: it releases the o_ps PSUM slot that gates
            # the next block's first O matmul, and must not queue behind
            # the DVE's accumulator backlog
            for kk in range(2):
                osl = slice(qt * QW + kk * 512, qt * QW + (kk + 1) * 512)
                nc.scalar.activation(o_sb[:, osl], ops[kk][:], AF.Copy)
            if qt == NQT - 1:
                # final block: PSUM s-slots are free now -- PE rowsum and
                # broadcast matmuls avoid the slow serial gpsimd chain.
                # Pipelined per 512-chunk to shorten the exposed tail.
                for c2 in range(2):
                    off = qt * QW + c2 * 512
                    sl = slice(off, off + 512)
                    dps = ps.tile([1, 512], f32, tag="s0")
                    nc.tensor.matmul(
                        dps[:], ones_c[:], acc[:, sl], start=True, stop=True
                    )
                    den_row = rrpool.tile([1, 512], f32, tag="drow")
                    nc.vector.tensor_copy(den_row[:], dps[:])
                    rs = rrpool.tile([C, 4], f32, tag="rs")
                    nc.sync.dma_start(rs[:], den_row[:])
                    rc = rrpool.tile([C, 4], bf16, tag="rc")
                    nc.vector.reciprocal(rc[:], rs[:])
                    rrow = rrpool.tile([1, 512], bf16, tag="rrow")
                    nc.sync.dma_start(rrow[:], rc[:])
                    bps = ps.tile([C, 512], f32, tag="s1")
                    nc.tensor.matmul(
                        bps[:], ones_r[:], rrow[:], start=True, stop=True
                    )
                    onrm = onrmp.tile([C, 512], bf16)
                    nc.vector.tensor_mul(onrm[:], o_sb[:, sl], bps[:])
                    pp = ps.tile([C, 512], f32, tag=f"pp{c2}")
                    nc.tensor.matmul(
                        pp[:], wp_b[:], onrm[:], start=True, stop=True
                    )
                    ost = ostg.tile([C, 512], f32)
                    nc.vector.tensor_add(ost[:], pp[:], x_sb[:, sl])
                    nc.sync.dma_start(out_dram[:, sl], ost[:])

        lp.__exit__(None, None, None)

    nc.compile()
    return nc


def _get_nc():
    if "nc" not in _CACHE:
        _CACHE["nc"] = _build()
    return _CACHE["nc"]


def _prep_inputs(x, gamma, beta, w_qkv, b_qkv, w_proj, b_proj):
    x = np.ascontiguousarray(x, dtype=np.float32)
    w_qkv = np.asarray(w_qkv, dtype=np.float32)
    b_qkv = np.asarray(b_qkv, dtype=np.float32)
    w_proj = np.asarray(w_proj, dtype=np.float32)
    b_proj = np.asarray(b_proj, dtype=np.float32)

    wq = w_qkv[0:C, :]
    wk = w_qkv[C:2 * C, :]
    wv = w_qkv[2 * C:3 * C, :]
    bqv = b_qkv[0:C]
    bkv = b_qkv[C:2 * C]
    bvv = b_qkv[2 * C:3 * C]

    wqT = np.ascontiguousarray((wq * SCALE).T)
    wkT = np.ascontiguousarray(wk.T)
    wvT = np.ascontiguousarray(wv.T)
    wpT = np.ascontiguousarray(w_proj.T)
    beff = (b_proj + w_proj @ bvv).astype(np.float32)

    ig = np.zeros((C, GROUPS), np.float32)
    ig[np.arange(C), np.arange(C) // GSIZE] = 1.0
    igt = np.ascontiguousarray(ig.T)
    ig = ig * (1.0 / GSIZE)  # fold the group-mean divide into the matmul

    common = {
        "gamma": np.asarray(gamma, np.float32).reshape(C, 1),
        "beta": np.asarray(beta, np.float32).reshape(C, 1),
        "bq": (bqv * SCALE).reshape(C, 1),
        "bk": bkv.reshape(C, 1),
        "beff": beff.reshape(C, 1),
        "wqT": wqT,
        "wkT": wkT,
        "wvT": wvT,
        "wpT": wpT,
        "ig": ig,
        "igt": igt,
    }
    in_maps = []
    for b in range(B):
        m = dict(common)
        m["x"] = np.ascontiguousarray(x[b].reshape(C, HW))
        in_maps.append(m)
    return in_maps


def kernel(x, gamma, beta, w_qkv, b_qkv, w_proj, b_proj):
    from concourse.bass_utils import run_bass_kernel_spmd

    nc = _get_nc()
    in_maps = _prep_inputs(x, gamma, beta, w_qkv, b_qkv, w_proj, b_proj)
    res = run_bass_kernel_spmd(nc, in_maps, list(range(B)))
    out = np.stack([res.results[b]["out"] for b in range(B)], axis=0)
    return out.reshape(B, C, H, W).astype(np.float32)


# revision 52
# speedup vs baseline: 1.0105x; 1.0005x over previous
"""Trainium2 Bass kernel for nn_AttentionBlock (GroupNorm + 1x1-conv QKV +
full self-attention over N=HW=4096 + output projection + residual).

Distribution: data-parallel over batch B=8, one batch element per NeuronCore.

v2: all matmul operands in bf16 (fp32 matmuls stream at ~2 cyc/col on TRN2
hardware -- "fp32_mode=HIGH/LOW" dual pass -- so bf16 doubles PE throughput).
The loop processes queries in 4 blocks of 1024 so exp runs as a single
[128,1024] ACT instruction per j-tile, with double-buffered S tiles in PSUM:

  per (block, j):  S^T = K_j^T Q_blk   (PE, 2x 512-col matmuls, bf16)
                   P^T = exp(S^T)      (ACT, PSUM->SBUF bf16)
                   O  += V_j^T P^T     (PE, 2x 512-col, PSUM accum over j)
                   acc += P^T          (DVE, bf16 pair tree)

Softmax denominators: bf16 pair-tree accumulation on DVE, cross-partition
sum via gpsimd partition_all_reduce (idle Pool engine), reciprocal on a
DMA-scattered [128,8] layout, broadcast back via gpsimd partition_broadcast.
Tail uses proj/normalize commutation: O is normalized per-query BEFORE the
projection (they commute since the denominator is a per-query scalar), so
out = (w_proj^T (O * recip)) + (x + b_eff) with the residual pre-folded.

Bias algebra: b_k folded into the K PSUM->SBUF copies (ACT bias); b_q into
the Q copies (DVE); b_v folded into b_eff = b_proj + w_proj @ b_v (host,
exact). The attention scale C^-0.5 is folded into w_q/b_q on the host.
No max-subtraction in softmax: logits are ~N(0,1) so fp32 exp is safe.
"""

import numpy as np

B, C, H, W = 8, 128, 64, 64
HW = H * W                      # 4096
GROUPS = 8
GSIZE = C // GROUPS             # 16
EPS = 1e-5
NJ = HW // 128                  # 32 j-tiles
QW = 1024                       # queries per block
NQT = HW // QW                  # 4 blocks
SCALE = float(C) ** -0.5

_CACHE = {}


def _build():
    from contextlib import ExitStack

    import concourse.bacc as bacc
    import concourse.tile as tile
    from concourse import bass_isa, mybir

    f32 = mybir.dt.float32
    bf16 = mybir.dt.bfloat16
    AF = mybir.ActivationFunctionType

    nc = bacc.Bacc("TRN2", target_bir_lowering=False, debug=False)

    x_in = nc.dram_tensor("x", [C, HW], f32, kind="ExternalInput")
    gamma_in = nc.dram_tensor("gamma", [C, 1], f32, kind="ExternalInput")
    beta_in = nc.dram_tensor("beta", [C, 1], f32, kind="ExternalInput")
    bq_in = nc.dram_tensor("bq", [C, 1], f32, kind="ExternalInput")
    bk_in = nc.dram_tensor("bk", [C, 1], f32, kind="ExternalInput")
    beff_in = nc.dram_tensor("beff", [C, 1], f32, kind="ExternalInput")
    wq_in = nc.dram_tensor("wqT", [C, C], f32, kind="ExternalInput")
    wk_in = nc.dram_tensor("wkT", [C, C], f32, kind="ExternalInput")
    wv_in = nc.dram_tensor("wvT", [C, C], f32, kind="ExternalInput")
    wp_in = nc.dram_tensor("wpT", [C, C], f32, kind="ExternalInput")
    ig_in = nc.dram_tensor("ig", [C, GROUPS], f32, kind="ExternalInput")
    igt_in = nc.dram_tensor("igt", [GROUPS, C], f32, kind="ExternalInput")
    out_dram = nc.dram_tensor("out", [C, HW], f32, kind="ExternalOutput")

    with tile.TileContext(nc) as tc, ExitStack() as ctx:
        const = ctx.enter_context(tc.tile_pool(name="const", bufs=1))
        big = ctx.enter_context(tc.tile_pool(name="big", bufs=1))
        stats = ctx.enter_context(tc.tile_pool(name="stats", bufs=1))
        ptpool = ctx.enter_context(tc.tile_pool(name="pt", bufs=24))
        tmpool = ctx.enter_context(tc.tile_pool(name="tmp", bufs=6))
        rrpool = ctx.enter_context(tc.tile_pool(name="rr", bufs=2))
        onrmp = ctx.enter_context(tc.tile_pool(name="onrm", bufs=3))
        ostg = ctx.enter_context(tc.tile_pool(name="ostg", bufs=3))
        ps = ctx.enter_context(tc.tile_pool(name="ps", bufs=1, space="PSUM"))

        # ---------------- load x (split over two DMA queues), consts ------
        NCH = 4
        CHW = HW // NCH  # 1024
        x_sb = big.tile([C, HW], f32, tag="x")
        xq = [nc.sync, nc.scalar, nc.gpsimd]
        for ch in range(8):
            sl = slice(ch * 512, (ch + 1) * 512)
            xq[ch % 3].dma_start(x_sb[:, sl], x_in[:, sl])

        def cload(t_in, shape, tag):
            t = const.tile(shape, f32, tag=tag)
            nc.sync.dma_start(t[:], t_in[:])
            return t

        gamma = cload(gamma_in, [C, 1], "c_gamma")
        beta = cload(beta_in, [C, 1], "c_beta")
        bq = cload(bq_in, [C, 1], "c_bq")
        bk = cload(bk_in, [C, 1], "c_bk")
        beff = cload(beff_in, [C, 1], "c_beff")
        ig = cload(ig_in, [C, GROUPS], "c_ig")
        igt = cload(igt_in, [GROUPS, C], "c_igt")
        wq_f = cload(wq_in, [C, C], "c_wq_f")
        wk_f = cload(wk_in, [C, C], "c_wk_f")
        wv_f = cload(wv_in, [C, C], "c_wv_f")
        wp_f = cload(wp_in, [C, C], "c_wp_f")

        with nc.allow_low_precision(reason="bf16 weights: rel tol is 2e-2"):
            wq_b = const.tile([C, C], bf16)
            nc.vector.tensor_copy(wq_b[:], wq_f[:])
            wk_b = const.tile([C, C], bf16)
            nc.vector.tensor_copy(wk_b[:], wk_f[:])
            wv_b = const.tile([C, C], bf16)
            nc.vector.tensor_copy(wv_b[:], wv_f[:])
            wp_b = const.tile([C, C], bf16)
            nc.vector.tensor_copy(wp_b[:], wp_f[:])

        ones_c = const.tile([C, 1], bf16)
        nc.vector.memset(ones_c[:], 1.0)
        ones_r = const.tile([1, C], bf16)
        nc.vector.memset(ones_r[:], 1.0)
        eps_t = const.tile([GROUPS, 1], f32)
        nc.vector.memset(eps_t[:], EPS)
        magic_t = const.tile([GROUPS, 1], mybir.dt.uint32)
        nc.vector.memset(magic_t[:], 0x5F3759DF)
        c15_t = const.tile([GROUPS, 1], f32)
        nc.vector.memset(c15_t[:], 1.5)

        # ---------------- groupnorm stats via bn_stats ----------------
        bnst = stats.tile([C, 8, 6], f32)
        for ch in range(8):
            sl = slice(ch * 512, (ch + 1) * 512)
            nc.vector.bn_stats(bnst[:, ch, :], x_sb[:, sl])
        mv = stats.tile([C, 2], f32)  # per-channel mean, var
        nc.vector.bn_aggr(mv[:], bnst[:])
        # warm the exp activation table before the loop
        warm = stats.tile([GROUPS, 1], f32)
        nc.scalar.activation(warm[:], eps_t[:], AF.Exp)

        # pack [mean, var + mean^2] -> group sums via indicator matmul
        msq = stats.tile([C, 2], f32)
        nc.vector.tensor_copy(msq[:, 0:1], mv[:, 0:1])
        nc.vector.tensor_mul(msq[:, 1:2], mv[:, 0:1], mv[:, 0:1])
        nc.vector.tensor_add(msq[:, 1:2], msq[:, 1:2], mv[:, 1:2])

        # ig is host-scaled by 1/GSIZE so the matmul yields means directly
        gs_ps = ps.tile([GROUPS, 2], f32, tag="pp0")
        nc.tensor.matmul(gs_ps[:], ig[:], msq[:], start=True, stop=True)
        gmr = stats.tile([GROUPS, 2], f32)
        nc.vector.tensor_copy(gmr[:, 0:1], gs_ps[:, 0:1])
        gmsq = stats.tile([GROUPS, 1], f32)
        nc.vector.tensor_mul(gmsq[:], gmr[:, 0:1], gmr[:, 0:1])
        gve = stats.tile([GROUPS, 1], f32)
        nc.vector.tensor_sub(gve[:], gs_ps[:, 1:2], gmsq[:])
        nc.vector.tensor_scalar(
            gve[:], gve[:], eps_t[:], None, mybir.AluOpType.add
        )
        # rstd = rsqrt(var+eps): quake guess + 1 Newton step (rel err
        # ~1.7e-3, far below the bf16 data-path noise)
        u32 = mybir.dt.uint32
        gu = stats.tile([GROUPS, 1], u32)
        nc.vector.tensor_scalar(
            gu[:], gve[:].bitcast(u32), 1, None,
            mybir.AluOpType.logical_shift_right,
        )
        nc.vector.tensor_sub(gu[:], magic_t[:], gu[:])
        gy = stats.tile([GROUPS, 1], f32)
        nc.vector.tensor_copy(gy[:], gu[:].bitcast(f32))
        gh = stats.tile([GROUPS, 1], f32)
        nc.vector.tensor_scalar_mul(gh[:], gve[:], 0.5)
        gt = stats.tile([GROUPS, 1], f32)
        nc.vector.tensor_mul(gt[:], gy[:], gy[:])
        nc.vector.tensor_mul(gt[:], gt[:], gh[:])
        nc.vector.tensor_sub(gt[:], c15_t[:], gt[:])
        nc.vector.tensor_mul(gmr[:, 1:2], gy[:], gt[:])

        bc_ps = ps.tile([C, 2], f32, tag="pp1")
        nc.tensor.matmul(bc_ps[:], igt[:], gmr[:], start=True, stop=True)
        a_c = stats.tile([C, 1], f32)
        b_c = stats.tile([C, 1], f32)
        tmc = stats.tile([C, 1], f32)
        nc.vector.tensor_scalar_mul(a_c[:], gamma[:], bc_ps[:, 1:2])
        nc.vector.tensor_scalar_mul(tmc[:], a_c[:], bc_ps[:, 0:1])
        nc.vector.tensor_sub(b_c[:], beta[:], tmc[:])

        # ---------------- hn (bf16) and QKV ----------------
        hn = big.tile([C, HW], bf16, tag="hn")
        q_r = big.tile([C, HW], bf16, tag="q")
        k_r = big.tile([C, HW], bf16, tag="k")
        vt = big.tile([C, NJ, 128], bf16, tag="vt")
        acc = big.tile([C, HW], bf16, tag="acc")
        o_sb = big.tile([C, HW], bf16, tag="o")

        lp = nc.allow_low_precision(reason="bf16 data path: rel tol is 2e-2")
        lp.__enter__()

        def emit_hn(ch):
            sl = slice(ch * CHW, (ch + 1) * CHW)
            nc.vector.tensor_scalar(
                hn[:, sl], x_sb[:, sl], a_c[:], b_c[:],
                mybir.AluOpType.mult, mybir.AluOpType.add,
            )

        emit_hn(0)  # chunk 0 unblocks K0/Q0/V0; the rest follow

        def emit_k_half(r, h):  # [C, 512] half on a pp bank
            kp = ps.tile([C, 512], f32, tag=f"pp{h}")
            off = r * QW + h * 512
            nc.tensor.matmul(
                kp[:], wk_b[:], hn[:, off:off + 512], start=True, stop=True
            )
            nc.scalar.activation(
                k_r[:, off:off + 512], kp[:], AF.Identity, bias=bk[:]
            )

        def emit_q_half(r, h, on_act=False):
            qp = ps.tile([C, 512], f32, tag=f"pp{h}")
            off = r * QW + h * 512
            nc.tensor.matmul(
                qp[:], wq_b[:], hn[:, off:off + 512], start=True, stop=True
            )
            if on_act:  # preamble only: ACT is idle there, DVE is not
                nc.scalar.activation(
                    q_r[:, off:off + 512], qp[:], AF.Identity, bias=bq[:]
                )
            else:
                nc.vector.tensor_scalar(
                    q_r[:, off:off + 512], qp[:], bq[:], None,
                    mybir.AluOpType.add,
                )

        def emit_v_half(r, h):  # 2 n-tiles, on a pp bank: fine-grained
            # emission keeps the PE surplus within the per-iter ACT slack
            vp = ps.tile([C, 2, C], f32, tag=f"pp{h}")
            for t in range(2):
                nt = r * 4 + h * 2 + t
                nc.tensor.matmul(
                    vp[:, t, :], hn[:, nt * 128:(nt + 1) * 128], wv_b[:],
                    start=True, stop=True,
                )
            base = r * 4 + h * 2
            nc.vector.tensor_copy(vt[:, base:base + 2, :], vp[:])

        v_nat = big.tile([C, HW], bf16, tag="vnat")
        vtq = [nc.sync, nc.scalar]

        def emit_v_chunk(ch):
            # V rounds 1-7 via one natural-layout matmul + xbar DMA
            # transposes on idle queues: 1 PE matmul instead of 4
            vnp = ps.tile([C, 512], f32, tag=f"pp{ch % 2}")
            sl = slice(ch * 512, (ch + 1) * 512)
            nc.tensor.matmul(vnp[:], wv_b[:], hn[:, sl], start=True, stop=True)
            nc.vector.tensor_copy(v_nat[:, sl], vnp[:])
            for t in range(4):
                nt = ch * 4 + t
                vtq[nt % 2].dma_start_transpose(
                    vt[:, nt, :], v_nat[:, nt * 128:(nt + 1) * 128]
                )

        # K/Q for the first 1024 queries/keys + the first V tiles before
        # the loop (biases on ACT/DVE: the Pool queue would serialize the
        # critical path here); the rest of K, V, and Q are interleaved
        # into block 0's j-loop with biases on the idle Pool engine
        emit_k_half(0, 0)
        emit_k_half(0, 1)
        emit_q_half(0, 0, on_act=True)
        emit_q_half(0, 1, on_act=True)
        emit_v_half(0, 0)
        emit_v_half(0, 1)
        for ch in range(1, NCH):
            emit_hn(ch)

        # ---------------- main attention loop ----------------
        # The softmax-denominator + projection chain for block qt-1 runs
        # entirely on PE/DVE/DMA, time-multiplexing the two pp PSUM banks
        # (rowsum -> broadcast -> projection), with stages pinned to well-
        # separated emission points in block qt's j-loop so no engine queue
        # ever blocks on a pending cross-engine chain.
        chain = {}

        def den_rowsum(qt, c2):
            sl = slice(qt * QW + c2 * 512, qt * QW + (c2 + 1) * 512)
            dps = ps.tile([1, 512], f32, tag=f"pp{c2}")
            nc.tensor.matmul(dps[:], ones_c[:], acc[:, sl], start=True,
                             stop=True)
            den_row = rrpool.tile([1, 512], f32, tag=f"drow{c2}")
            nc.vector.tensor_copy(den_row[:], dps[:])
            rs = rrpool.tile([C, 4], f32, tag=f"rs{c2}")
            nc.sync.dma_start(rs[:], den_row[:])
            chain[f"rs{c2}"] = rs

        def den_recip(c2):
            rc = rrpool.tile([C, 4], bf16, tag=f"rc{c2}")
            nc.vector.reciprocal(rc[:], chain[f"rs{c2}"][:])
            rrow = rrpool.tile([1, 512], bf16, tag=f"rrow{c2}")
            nc.sync.dma_start(rrow[:], rc[:])
            chain[f"rrow{c2}"] = rrow

        def den_bcast(c2):
            bps = ps.tile([C, 512], f32, tag=f"pp{c2}")
            nc.tensor.matmul(bps[:], ones_r[:], chain[f"rrow{c2}"][:],
                             start=True, stop=True)
            chain[f"bps{c2}"] = bps

        def proj_onrm(qt, c2):
            sl = slice(qt * QW + c2 * 512, qt * QW + (c2 + 1) * 512)
            onrm = onrmp.tile([C, 512], bf16)
            nc.vector.tensor_mul(onrm[:], o_sb[:, sl], chain[f"bps{c2}"][:])
            chain[f"onrm{c2}"] = onrm

        def proj_store(qt, c2):
            sl = slice(qt * QW + c2 * 512, qt * QW + (c2 + 1) * 512)
            pp = ps.tile([C, 512], f32, tag=f"pp{c2}")
            nc.tensor.matmul(pp[:], wp_b[:], chain[f"onrm{c2}"][:],
                             start=True, stop=True)
            ost = ostg.tile([C, 512], f32)
            nc.vector.tensor_add(ost[:], pp[:], x_sb[:, sl])
            nc.sync.dma_start(out_dram[:, sl], ost[:])

        for qt in range(NQT):
            qsl = slice(qt * QW, (qt + 1) * QW)
            op0 = ps.tile([C, 512], f32, tag="o0")
            op1 = ps.tile([C, 512], f32, tag="o1")
            ops = [op0, op1]
            pts = [None, None]
            for j in range(NJ):
                if qt == 0:
                    if j % 4 == 2 and j <= 26:
                        emit_v_half(j // 4 + 1, 0)
                    if j % 4 == 0 and 4 <= j <= 28:
                        emit_v_half(j // 4, 1)
                    if j % 8 == 5 and j < 24:
                        r = j // 8 + 1
                        emit_k_half(r, 0)
                        emit_k_half(r, 1)
                if qt in (1, 2) and j == 5:
                    # residual+bias prefold: x_sb <- x + beff, placed in the
                    # DVE-slack quarters (chunk qt-1 feeds ost at j15)
                    sl2 = slice((qt - 1) * 2048, qt * 2048)
                    nc.vector.tensor_scalar(
                        x_sb[:, sl2], x_sb[:, sl2], beff[:], None,
                        mybir.AluOpType.add,
                    )
                if qt > 0:
                    if j == 3:
                        den_rowsum(qt - 1, 0)
                    elif j == 4:
                        den_rowsum(qt - 1, 1)
                    elif j == 7:
                        den_recip(0)
                    elif j == 8:
                        den_recip(1)
                    elif j == 11:
                        den_bcast(0)
                    elif j == 12:
                        den_bcast(1)
                    elif j == 13:
                        proj_onrm(qt - 1, 0)
                    elif j == 14:
                        proj_onrm(qt - 1, 1)
                    elif j == 15:
                        proj_store(qt - 1, 0)
                    elif j == 16:
                        proj_store(qt - 1, 1)
                if j == 17 and qt < NQT - 1:
                    emit_q_half(qt + 1, 0)
                if j == 19 and qt < NQT - 1:
                    emit_q_half(qt + 1, 1)
                sp = ps.tile([C, QW], f32, tag=f"s{j % 2}")
                for kk in range(2):
                    qoff = qt * QW + kk * 512
                    nc.tensor.matmul(
                        sp[:, kk * 512:(kk + 1) * 512],
                        k_r[:, j * 128:(j + 1) * 128],
                        q_r[:, qoff:qoff + 512],
                        start=True, stop=True,
                    )
                pt = ptpool.tile([C, QW], bf16)
                nc.scalar.activation(pt[:], sp[:], AF.Exp)
                for kk in range(2):
                    sl = slice(kk * 512, (kk + 1) * 512)
                    nc.tensor.matmul(
                        ops[kk][:], vt[:, j, :], pt[:, sl],
                        start=(j == 0), stop=(j == NJ - 1),
                    )
                pts[j % 2] = pt
                if j % 2 == 1:
                    if j == 1:
                        nc.vector.tensor_add(acc[:, qsl], pts[0][:], pts[1][:])
                    else:
                        tmp = tmpool.tile([C, QW], bf16)
                        nc.vector.tensor_add(tmp[:], pts[0][:], pts[1][:])
                        nc.vector.tensor_add(acc[:, qsl], acc[:, qsl], tmp[:])

            # ---- block epilogue ----
            # o_sb copy on ACT: it releases the o_ps PSUM slot that gates
            # the next block's first O matmul, and must not queue behind
            # the DVE's accumulator backlog
            for kk in range(2):
                osl = slice(qt * QW + kk * 512, qt * QW + (kk + 1) * 512)
                nc.scalar.activation(o_sb[:, osl], ops[kk][:], AF.Copy)
            if qt == NQT - 1:
                # final block: PSUM s-slots are free now -- PE rowsum and
                # broadcast matmuls avoid the slow serial gpsimd chain.
                # Pipelined per 512-chunk to shorten the exposed tail.
                for c2 in range(2):
                    off = qt * QW + c2 * 512
                    sl = slice(off, off + 512)
                    dps = ps.tile([1, 512], f32, tag="s0")
                    nc.tensor.matmul(
                        dps[:], ones_c[:], acc[:, sl], start=True, stop=True
                    )
                    den_row = rrpool.tile([1, 512], f32, tag="drow")
                    nc.vector.tensor_copy(den_row[:], dps[:])
                    rs = rrpool.tile([C, 4], f32, tag="rs")
                    nc.sync.dma_start(rs[:], den_row[:])
                    rc = rrpool.tile([C, 4], bf16, tag="rc")
                    nc.vector.reciprocal(rc[:], rs[:])
                    rrow = rrpool.tile([1, 512], bf16, tag="rrow")
                    nc.sync.dma_start(rrow[:], rc[:])
                    bps = ps.tile([C, 512], f32, tag="s1")
                    nc.tensor.matmul(
                        bps[:], ones_r[:], rrow[:], start=True, stop=True
                    )
                    onrm = onrmp.tile([C, 512], bf16)
                    nc.vector.tensor_mul(onrm[:], o_sb[:, sl], bps[:])
                    pp = ps.tile([C, 512], f32, tag=f"pp{c2}")
                    nc.tensor.matmul(
                        pp[:], wp_b[:], onrm[:], start=True, stop=True
                    )
                    ost = ostg.tile([C, 512], f32)
                    nc.vector.tensor_add(ost[:], pp[:], x_sb[:, sl])
                    nc.sync.dma_start(out_dram[:, sl], ost[:])

        lp.__exit__(None, None, None)

    nc.compile()
    return nc


def _get_nc():
    if "nc" not in _CACHE:
        _CACHE["nc"] = _build()
    return _CACHE["nc"]


def _prep_inputs(x, gamma, beta, w_qkv, b_qkv, w_proj, b_proj):
    x = np.ascontiguousarray(x, dtype=np.float32)
    w_qkv = np.asarray(w_qkv, dtype=np.float32)
    b_qkv = np.asarray(b_qkv, dtype=np.float32)
    w_proj = np.asarray(w_proj, dtype=np.float32)
    b_proj = np.asarray(b_proj, dtype=np.float32)

    wq = w_qkv[0:C, :]
    wk = w_qkv[C:2 * C, :]
    wv = w_qkv[2 * C:3 * C, :]
    bqv = b_qkv[0:C]
    bkv = b_qkv[C:2 * C]
    bvv = b_qkv[2 * C:3 * C]

    wqT = np.ascontiguousarray((wq * SCALE).T)
    wkT = np.ascontiguousarray(wk.T)
    wvT = np.ascontiguousarray(wv.T)
    wpT = np.ascontiguousarray(w_proj.T)
    beff = (b_proj + w_proj @ bvv).astype(np.float32)

    ig = np.zeros((C, GROUPS), np.float32)
    ig[np.arange(C), np.arange(C) // GSIZE] = 1.0
    igt = np.ascontiguousarray(ig.T)
    ig = ig * (1.0 / GSIZE)  # fold the group-mean divide into the matmul

    common = {
        "gamma": np.asarray(gamma, np.float32).reshape(C, 1),
        "beta": np.asarray(beta, np.float32).reshape(C, 1),
        "bq": (bqv * SCALE).reshape(C, 1),
        "bk": bkv.reshape(C, 1),
        "beff": beff.reshape(C, 1),
        "wqT": wqT,
        "wkT": wkT,
        "wvT": wvT,
        "wpT": wpT,
        "ig": ig,
        "igt": igt,
    }
    in_maps = []
    for b in range(B):
        m = dict(common)
        m["x"] = np.ascontiguousarray(x[b].reshape(C, HW))
        in_maps.append(m)
    return in_maps


def kernel(x, gamma, beta, w_qkv, b_qkv, w_proj, b_proj):
    from concourse.bass_utils import run_bass_kernel_spmd

    nc = _get_nc()
    in_maps = _prep_inputs(x, gamma, beta, w_qkv, b_qkv, w_proj, b_proj)
    res = run_bass_kernel_spmd(nc, in_maps, list(range(B)))
    out = np.stack([res.results[b]["out"] for b in range(B)], axis=0)
    return out.reshape(B, C, H, W).astype(np.float32)
